# revision 2
# baseline (speedup 1.0000x reference)
"""Trainium2 Bass kernel for a GQA causal attention block (B=2, S=2048,
HID=2048, 16 q-heads / 4 kv-heads, RoPE, causal softmax, output proj).

Sharding: core c in [0,8) handles batch b = c//4 and head-group g = c%4
(q-heads 4g..4g+3, kv-head g).  Wq/Wk/Wv are column-sharded by head group,
Wo row-sharded; each core emits a partial output and the host sums the 4
partials per batch.

Per-core kernel (all matmuls free-dim 512 where possible, bf16 inputs with
fp32 PSUM accumulation):
  - qT/kT computed in [d, s] layout directly (weights pre-transposed on
    host); RoPE applied in rotate-half form (weight rows pre-permuted
    evens-then-odds on host) via DVE ops on [64, 512] tiles.
  - scores computed TRANSPOSED, sT[k, q] = kT.T-tile @ qT, so the PV matmul
    consumes exp(sT) directly with no on-chip transposes.
  - softmax without max subtraction (scores ~N(0, 0.8); exp is safe in f32),
    denominator accumulated in f32 SBUF and reduced with a ones-matmul,
    normalization broadcast via a K=1 matmul + DVE multiply.
"""

import numpy as np
import ml_dtypes

try:
    import concourse  # noqa: F401
except ImportError:  # pragma: no cover - path fallback
    import sys

    for _p in ("/root/.axon_site/_ro/trn_rl_repo", "/opt/trn_rl_repo"):
        if _p not in sys.path:
            sys.path.append(_p)

from contextlib import ExitStack

import concourse.bass as bass
import concourse.tile as tile
from concourse import bacc, mybir
from concourse.bass_utils import run_bass_kernel_spmd

F32 = mybir.dt.float32
BF16 = mybir.dt.bfloat16

B = 2
S = 2048
HID = 2048
HEADS = 16
KV_HEADS = 4
HD = 128
HALF = HD // 2
QH = HEADS // KV_HEADS  # q heads per core (4)
LO = QH * HD  # local q/o width (512)
N_CORES = 8

NEG = -1.0e5  # additive causal mask value (exp -> exactly 0 in f32)


def _emit(ctx: ExitStack, tc: "tile.TileContext", aps: dict, s_len: int):
    nc = tc.nc
    IT = HID // 128  # contraction tiles (16)
    SC = s_len // 512  # s-chunks of 512
    KBT = s_len // 128  # 128-wide k blocks
    QBT = s_len // 512  # 512-wide q blocks

    xT, wqT, wkT, wvT, woT = aps["xT"], aps["wqT"], aps["wkT"], aps["wvT"], aps["woT"]
    cosq, sinq, cosk, sink = aps["cosq"], aps["sinq"], aps["cosk"], aps["sink"]
    mtri, outp = aps["mtri"], aps["outp"]

    # ---- pools ----
    xpool = ctx.enter_context(tc.tile_pool(name="xpool", bufs=2))
    spsum = ctx.enter_context(tc.tile_pool(name="spsum", bufs=3, space="PSUM"))
    ypsum = ctx.enter_context(tc.tile_pool(name="ypsum", bufs=2, space="PSUM"))
    lpsum = ctx.enter_context(tc.tile_pool(name="lpsum", bufs=2, space="PSUM"))
    bpsum = ctx.enter_context(tc.tile_pool(name="bpsum", bufs=1, space="PSUM"))
    ptpool = ctx.enter_context(tc.tile_pool(name="ptpool", bufs=3))
    accpool = ctx.enter_context(tc.tile_pool(name="accpool", bufs=2))
    ropet = ctx.enter_context(tc.tile_pool(name="ropet", bufs=4))
    bcpool = ctx.enter_context(tc.tile_pool(name="bcpool", bufs=2))
    invpool = ctx.enter_context(tc.tile_pool(name="invpool", bufs=2))
    outpool = ctx.enter_context(tc.tile_pool(name="outpool", bufs=3))

    # ---- persistent SBUF tensors ----
    def single(shape, dtype, name):
        t, free = tc.tile(shape, dtype, name=name)
        ctx.callback(free)
        return t

    wq_sb = single([128, IT, LO], BF16, "wq_sb")
    wk_sb = single([128, IT, HD], BF16, "wk_sb")
    wv_sb = single([128, IT, HD], BF16, "wv_sb")
    wo_sb = single([128, QH, HID], BF16, "wo_sb")
    cq_sb = single([HALF, s_len], F32, "cq_sb")
    sq_sb = single([HALF, s_len], F32, "sq_sb")
    ck_sb = single([HALF, s_len], F32, "ck_sb")
    sk_sb = single([HALF, s_len], F32, "sk_sb")
    mtri_sb = single([128, 128], F32, "mtri_sb")
    qT_sb = single([128, QH, s_len], BF16, "qT_sb")
    kT_sb = single([128, s_len], BF16, "kT_sb")
    v_sb = single([128, KBT, HD], BF16, "v_sb")
    yT_sb = single([128, QH, s_len], BF16, "yT_sb")
    ones_col = single([128, 1], F32, "ones_col")
    ones_row = single([1, 128], F32, "ones_row")

    nc.vector.memset(ones_col, 1.0)
    nc.vector.memset(ones_row, 1.0)

    nc.sync.dma_start(out=wq_sb, in_=wqT.rearrange("(it p) o -> p it o", p=128))
    nc.sync.dma_start(out=wk_sb, in_=wkT.rearrange("(it p) o -> p it o", p=128))
    nc.sync.dma_start(out=wv_sb, in_=wvT.rearrange("(it p) o -> p it o", p=128))
    nc.sync.dma_start(out=wo_sb, in_=woT.rearrange("(jt p) o -> p jt o", p=128))
    nc.sync.dma_start(out=cq_sb, in_=cosq)
    nc.sync.dma_start(out=sq_sb, in_=sinq)
    nc.sync.dma_start(out=ck_sb, in_=cosk)
    nc.sync.dma_start(out=sk_sb, in_=sink)
    nc.sync.dma_start(out=mtri_sb, in_=mtri)

    xT_r = xT.rearrange("(it p) s -> p it s", p=128)

    def rope(ps, out_top, out_bot, cos_sb, sin_sb, sc):
        """out = RoPE(ps) in rotate-half layout; ps is a [128, 512] psum tile
        whose partitions are [evens(64); odds(64)] of one head."""
        cs = cos_sb[:, sc * 512 : (sc + 1) * 512]
        sn = sin_sb[:, sc * 512 : (sc + 1) * 512]
        top = ps[0:HALF, :]
        bot = ps[HALF:128, :]
        t1 = ropet.tile([HALF, 512], F32, tag="t1", name="t1")
        t2 = ropet.tile([HALF, 512], F32, tag="t2", name="t2")
        nc.vector.tensor_mul(t1, top, cs)
        nc.vector.tensor_mul(t2, bot, sn)
        nc.vector.tensor_sub(out_top, t1, t2)
        t3 = ropet.tile([HALF, 512], F32, tag="t3", name="t3")
        t4 = ropet.tile([HALF, 512], F32, tag="t4", name="t4")
        nc.vector.tensor_mul(t3, top, sn)
        nc.vector.tensor_mul(t4, bot, cs)
        nc.vector.tensor_add(out_bot, t3, t4)

    # ---------------- phase 1: Q/K/V projections + RoPE ----------------
    for sc in range(SC):
        xs = xpool.tile([128, IT, 512], BF16, tag="xs", name="xs")
        nc.sync.dma_start(out=xs, in_=xT_r[:, :, sc * 512 : (sc + 1) * 512])

        # Q: per head, accumulate over i-tiles -> [128(d), 512(s)] psum
        for h in range(QH):
            ps_q = spsum.tile([128, 512], F32, tag="ps", name="ps_q")
            for it in range(IT):
                nc.tensor.matmul(
                    ps_q,
                    wq_sb[:, it, h * HD : (h + 1) * HD],
                    xs[:, it, :],
                    start=(it == 0),
                    stop=(it == IT - 1),
                )
            sl = slice(sc * 512, (sc + 1) * 512)
            rope(ps_q, qT_sb[0:HALF, h, sl], qT_sb[HALF:128, h, sl], cq_sb, sq_sb, sc)

        # K: one kv head
        ps_k = spsum.tile([128, 512], F32, tag="ps", name="ps_k")
        for it in range(IT):
            nc.tensor.matmul(
                ps_k,
                wk_sb[:, it, :],
                xs[:, it, :],
                start=(it == 0),
                stop=(it == IT - 1),
            )
        sl = slice(sc * 512, (sc + 1) * 512)
        rope(ps_k, kT_sb[0:HALF, sl], kT_sb[HALF:128, sl], ck_sb, sk_sb, sc)

        # V: natural [s, d] layout, four 128-row s-tiles per chunk
        for sj in range(4):
            st = sc * 4 + sj
            ps_v = spsum.tile([128, 512], F32, tag="ps", name="ps_v")
            for it in range(IT):
                nc.tensor.matmul(
                    ps_v[:, 0:HD],
                    xs[:, it, sj * 128 : (sj + 1) * 128],
                    wv_sb[:, it, :],
                    start=(it == 0),
                    stop=(it == IT - 1),
                )
            nc.scalar.copy(v_sb[:, st, :], ps_v[:, 0:HD])

    # ---------------- phase 2: attention ----------------
    pending = []  # deferred per-(h,J) finalizers, emitted after the next
    # block's first scores matmul to keep PE busy across the boundary

    def flush_pending():
        while pending:
            pending.pop(0)()

    for h in range(QH):
        for J in range(QBT):
            nkb = 4 * J + 4
            qsl = slice(J * 512, (J + 1) * 512)

            ps_y = ypsum.tile([128, 512], F32, tag="ps_y", name="ps_y")
            pacc = accpool.tile([128, 512], F32, tag="pacc", name="pacc")

            for kb in range(nkb):
                r = kb - 4 * J  # >=0 on diagonal blocks
                lo = r * 128 if r >= 0 else 0

                ps_s = spsum.tile([128, 512], F32, tag="ps", name="ps_s")
                nc.tensor.matmul(
                    ps_s[:, lo:512],
                    kT_sb[:, kb * 128 : (kb + 1) * 128],
                    qT_sb[:, h, J * 512 + lo : (J + 1) * 512],
                    start=True,
                    stop=True,
                )
                if kb == 1:
                    flush_pending()
                if r >= 0:
                    nc.vector.tensor_add(
                        ps_s[:, lo : lo + 128], ps_s[:, lo : lo + 128], mtri_sb
                    )
                pt = ptpool.tile([128, 512], BF16, tag="pt", name="pt")
                nc.scalar.activation(
                    pt[:, lo:512], ps_s[:, lo:512], mybir.ActivationFunctionType.Exp
                )
                if kb == 0:
                    nc.vector.tensor_copy(pacc[:, lo:512], pt[:, lo:512])
                else:
                    nc.vector.tensor_add(
                        pacc[:, lo:512], pacc[:, lo:512], pt[:, lo:512]
                    )
                nc.tensor.matmul(
                    ps_y[:, lo:512],
                    v_sb[:, kb, :],
                    pt[:, lo:512],
                    start=(kb == 0),
                    stop=(kb == nkb - 1),
                )

            def finalize(h=h, J=J, ps_y=ps_y, pacc=pacc, qsl=qsl):
                ps_l = lpsum.tile([1, 512], F32, tag="ps_l", name="ps_l")
                nc.tensor.matmul(ps_l, ones_col, pacc, start=True, stop=True)
                inv = invpool.tile([1, 512], F32, tag="inv", name="inv")
                nc.vector.reciprocal(inv, ps_l)
                ps_b = bpsum.tile([128, 512], F32, tag="ps_b", name="ps_b")
                nc.tensor.matmul(ps_b, ones_row, inv, start=True, stop=True)
                bc = bcpool.tile([128, 512], F32, tag="bc", name="bc")
                nc.scalar.copy(bc, ps_b)
                nc.vector.tensor_mul(yT_sb[:, h, qsl], ps_y, bc)

            pending.append(finalize)

    flush_pending()

    # ---------------- phase 3: output projection ----------------
    for st in range(KBT):
        for ob in range(HID // 512):
            ps_o = ypsum.tile([128, 512], F32, tag="ps_y", name="ps_o")
            for h in range(QH):
                nc.tensor.matmul(
                    ps_o,
                    yT_sb[:, h, st * 128 : (st + 1) * 128],
                    wo_sb[:, h, ob * 512 : (ob + 1) * 512],
                    start=(h == 0),
                    stop=(h == QH - 1),
                )
            o_sb = outpool.tile([128, 512], F32, tag="o_sb", name="o_sb")
            nc.scalar.copy(o_sb, ps_o)
            nc.sync.dma_start(
                out=outp[st * 128 : (st + 1) * 128, ob * 512 : (ob + 1) * 512],
                in_=o_sb,
            )


def build_module(s_len: int = S):
    nc = bacc.Bacc(
        "TRN2", target_bir_lowering=False, debug=False, enable_asserts=False
    )
    aps = {}
    aps["xT"] = nc.dram_tensor("xT", [HID, s_len], BF16, kind="ExternalInput").ap()
    aps["wqT"] = nc.dram_tensor("wqT", [HID, LO], BF16, kind="ExternalInput").ap()
    aps["wkT"] = nc.dram_tensor("wkT", [HID, HD], BF16, kind="ExternalInput").ap()
    aps["wvT"] = nc.dram_tensor("wvT", [HID, HD], BF16, kind="ExternalInput").ap()
    aps["woT"] = nc.dram_tensor("woT", [LO, HID], BF16, kind="ExternalInput").ap()
    aps["cosq"] = nc.dram_tensor("cosq", [HALF, s_len], F32, kind="ExternalInput").ap()
    aps["sinq"] = nc.dram_tensor("sinq", [HALF, s_len], F32, kind="ExternalInput").ap()
    aps["cosk"] = nc.dram_tensor("cosk", [HALF, s_len], F32, kind="ExternalInput").ap()
    aps["sink"] = nc.dram_tensor("sink", [HALF, s_len], F32, kind="ExternalInput").ap()
    aps["mtri"] = nc.dram_tensor("mtri", [128, 128], F32, kind="ExternalInput").ap()
    aps["outp"] = nc.dram_tensor("outp", [s_len, HID], F32, kind="ExternalOutput").ap()

    with tile.TileContext(nc) as tc:
        with ExitStack() as ctx:
            _emit(ctx, tc, aps, s_len)
    nc.compile()
    return nc


_MODULE_CACHE: dict = {}


def _get_module(s_len: int = S):
    if s_len not in _MODULE_CACHE:
        _MODULE_CACHE[s_len] = build_module(s_len)
    return _MODULE_CACHE[s_len]


_PERM = np.concatenate([np.arange(0, HD, 2), np.arange(1, HD, 2)])  # evens|odds


def make_in_maps(x, cos, sin, Wq, Wk, Wv, Wo, s_len: int = S):
    """Build the 8 per-core input maps (host-side sharding + layout prep)."""
    x = np.asarray(x, dtype=np.float32)
    cos = np.asarray(cos, dtype=np.float32)
    sin = np.asarray(sin, dtype=np.float32)
    Wq = np.asarray(Wq, dtype=np.float32)
    Wk = np.asarray(Wk, dtype=np.float32)
    Wv = np.asarray(Wv, dtype=np.float32)
    Wo = np.asarray(Wo, dtype=np.float32)

    bf = ml_dtypes.bfloat16
    scale = 1.0 / np.sqrt(HD)

    cosT = np.ascontiguousarray(cos.T)  # [64, S]
    sinT = np.ascontiguousarray(sin.T)
    cosq = (cosT * scale).astype(np.float32)
    sinq = (sinT * scale).astype(np.float32)

    kk, qq = np.meshgrid(np.arange(128), np.arange(128), indexing="ij")
    mtri = np.where(kk <= qq, 0.0, NEG).astype(np.float32)

    Wq4 = Wq.reshape(HEADS, HD, HID)
    Wk4 = Wk.reshape(KV_HEADS, HD, HID)
    Wv4 = Wv.reshape(KV_HEADS, HD, HID)

    in_maps = []
    for c in range(N_CORES):
        b, g = divmod(c, KV_HEADS)
        hs = [g * QH + i for i in range(QH)]
        wq_l = Wq4[hs][:, _PERM, :].reshape(LO, HID)  # [512, 2048]
        wk_l = Wk4[g][_PERM, :]  # [128, 2048]
        wv_l = Wv4[g]  # [128, 2048]
        jcols = np.concatenate([np.arange(h * HD, (h + 1) * HD) for h in hs])
        wo_l = Wo[:, jcols]  # [2048, 512]

        in_maps.append(
            {
                "xT": np.ascontiguousarray(x[b].T).astype(bf),
                "wqT": np.ascontiguousarray(wq_l.T).astype(bf),
                "wkT": np.ascontiguousarray(wk_l.T).astype(bf),
                "wvT": np.ascontiguousarray(wv_l.T).astype(bf),
                "woT": np.ascontiguousarray(wo_l.T).astype(bf),
                "cosq": cosq,
                "sinq": sinq,
                "cosk": cosT,
                "sink": sinT,
                "mtri": mtri,
            }
        )
    return in_maps


def combine_outputs(results):
    out = np.zeros((B, S, HID), dtype=np.float32)
    for c in range(N_CORES):
        b = c // KV_HEADS
        out[b] += results[c]["outp"]
    return out


def kernel(x, cos, sin, Wq, Wk, Wv, Wo):
    nc = _get_module(S)
    in_maps = make_in_maps(x, cos, sin, Wq, Wk, Wv, Wo, S)
    res = run_bass_kernel_spmd(nc, in_maps, core_ids=list(range(N_CORES)))
    return combine_outputs(res.results)


def run_traced(x, cos, sin, Wq, Wk, Wv, Wo, **trace_kwargs):
    """Like kernel() but with NTFF tracing; returns (output, BassKernelResults)."""
    nc = _get_module(S)
    in_maps = make_in_maps(x, cos, sin, Wq, Wk, Wv, Wo, S)
    res = run_bass_kernel_spmd(
        nc, in_maps, core_ids=list(range(N_CORES)), trace=True, **trace_kwargs
    )
    return combine_outputs(res.results), res
